# revision 10
# baseline (speedup 1.0000x reference)
"""HBV hydrological model on Trainium2, 8 NeuronCores.

Scan-based parallel-in-time formulation (no 730-step serial loop):
  - snow pack: max-plus scans for SWE + exact "rainflow" clamp-composition
    scans for meltwater (iterated for the swe<->cap coupling)
  - soil moisture: frozen-branch linear scans, Picard + Newton steps
    (pow via ACT ln/exp over the full series); state scaled by 1/FC
  - upper zone: exact via exponential rescaling (chunks of 73 steps) +
    max-plus scans
  - lower zone: exact (add,mult) scan
Layout per core: 1250 cells -> 1280 = 128 partitions x 10 chains,
series [128, 7300] chain-major (free = chain*730 + t).
"""

import numpy as np
import ml_dtypes

import concourse.bass as bass
import concourse.mybir as mybir
from concourse.bass_utils import run_bass_kernel_spmd

Alu = mybir.AluOpType
Act = mybir.ActivationFunctionType
F32 = mybir.dt.float32
BF16 = mybir.dt.bfloat16

NSTEP = 730
NGRID = 10000
NCORE = 8
CELLS = NGRID // NCORE          # 1250
NCH = 10
PADG = 128 * NCH                # 1280
FREE = NSTEP * NCH              # 7300
CHUNK = 73
NCHUNK = NSTEP // CHUNK
INF = 1e30

SNOW_ITERS = 3
SM_SCHED = "PNNN"

# parameter bounds: BETA, FC, K0, K1, K2, LP, PERC, UZL, TT, CFMAX, CFR, CWH
BOUNDS = np.array([[1.0, 6.0], [50.0, 1000.0], [0.05, 0.9], [0.01, 0.5],
                   [0.001, 0.2], [0.2, 1.0], [0.0, 10.0], [0.0, 100.0],
                   [-2.5, 2.5], [0.5, 10.0], [0.0, 0.1], [0.0, 0.2]],
                  dtype=np.float32)

PARAM_NAMES = ["cwh", "beta", "invfc", "invlp", "fc", "lp", "pmax", "uzl",
               "k0", "k1", "kq2", "omk2", "sig0"]

LAST_RESULT = None
_NC = None


class Sched:
    """Per-engine in-order op queues; cross-engine deps via one semaphore
    per engine (every instruction increments its engine's semaphore).
    DMAs live on the sync queue in insertion order; each DMA completion
    bumps dsem by 16."""

    def __init__(self):
        self.ops = {e: [] for e in "vap"}
        self.counts = {e: 0 for e in "vap"}
        self.dmas = []
        self.n_dma = 0
        self.stopped = False

    def add(self, eng, emit, deps=()):
        if self.stopped:
            return None
        waits = {}
        for d in deps:
            if d is None:
                continue
            de, dc = d
            if de != eng:
                waits[de] = max(waits.get(de, 0), dc)
        self.ops[eng].append((emit, waits))
        self.counts[eng] += 1
        return (eng, self.counts[eng])

    def dma(self, dst, src, deps=()):
        if self.stopped:
            return None
        self.dmas.append((dst, src, [d for d in deps if d]))
        self.n_dma += 1
        return ("d", self.n_dma)


def _build(tap=None):
    nc = bass.Bass()
    din = {}
    for name in ["snowf", "dnet", "rainf", "petf", "kpe"]:
        din[name] = nc.declare_dram_parameter(name, [128, FREE], BF16,
                                              isOutput=False)
    for name in ["gsc", "hsc"]:
        din[name] = nc.declare_dram_parameter(name, [128, FREE], F32,
                                              isOutput=False)
    d_prm = nc.declare_dram_parameter("prm", [128, len(PARAM_NAMES) * NCH],
                                      F32, isOutput=False)
    if tap is None:
        d_swe = nc.declare_dram_parameter("swe", [128, FREE], F32,
                                          isOutput=True)
        d_aet = nc.declare_dram_parameter("aet", [128, FREE], BF16,
                                          isOutput=True)
        d_q = nc.declare_dram_parameter("q", [128, FREE], BF16, isOutput=True)
    else:
        d_dbg16 = nc.declare_dram_parameter("dbg16", [128, FREE], BF16,
                                            isOutput=True)
        d_dbg32 = nc.declare_dram_parameter("dbg32", [128, FREE], F32,
                                            isOutput=True)

    from contextlib import ExitStack
    with ExitStack() as st:
        B = [st.enter_context(nc.sbuf_tensor(f"bb{i}", [128, FREE], BF16))
             for i in range(6)]
        F = [st.enter_context(nc.sbuf_tensor(f"ff{i}", [128, FREE], F32))
             for i in range(4)]
        PRM = st.enter_context(
            nc.sbuf_tensor("prm_s", [128, len(PARAM_NAMES) * NCH], F32))
        ZERO = st.enter_context(nc.sbuf_tensor("zero_s", [128, 1], F32))
        CAR = st.enter_context(
            nc.sbuf_tensor("car_s", [128, NCHUNK * NCH], F32))
        PS = st.enter_context(nc.psum_tensor("ps_r", [128, NSTEP], F32))
        EPS = st.enter_context(nc.sbuf_tensor("eps_s", [128, 1], F32))

        sem = {e: st.enter_context(nc.semaphore(f"sem_{e}"))
               for e in "vap"}
        dsem = st.enter_context(nc.semaphore("dsem"))
        block = st.enter_context(nc.Block())

        def prm_col(name):
            i = PARAM_NAMES.index(name)
            return PRM[:, i * NCH:(i + 1) * NCH]

        def pb(name):
            return prm_col(name).unsqueeze(2).broadcast_to([128, NCH, NSTEP])

        def v3(ap):
            return ap.rearrange("p (j t) -> p j t", j=NCH)

        def cols0(buf):
            return v3(buf[:])[:, :, 0]

        def shifted(buf):
            return buf[:, 0:FREE - 1]

        def body(buf):
            return buf[:, 1:FREE]

        S = Sched()

        def _tap(name, ap, deps, f32=False):
            if tap == name and not S.stopped:
                S.dma(d_dbg32[:] if f32 else d_dbg16[:], ap, deps)
                S.stopped = True

        v = nc.vector
        gp = nc.gpsimd
        sc = nc.scalar

        # ---------------- DMA in: params + snow inputs ----------------
        t_prm = S.dma(PRM[:], d_prm[:])
        t_dnet = S.dma(B[1][:], din["dnet"][:])
        t_snowf = S.dma(B[0][:], din["snowf"][:])

        zer_t = S.add("v", lambda: v.memset(ZERO[:], 0.0))
        eps_t = S.add("v", lambda: v.memset(EPS[:], 1e-8))
        zb = ZERO[:].broadcast_to([128, NSTEP])

        # ================= SNOW =================
        Ap = F[0]
        t_A = []
        for j in range(NCH):
            sl = slice(j * NSTEP, (j + 1) * NSTEP)
            t_A.append(S.add("v", (lambda sl=sl: v.tensor_tensor_scan(
                Ap[:, sl], zb, B[1][:, sl], 0.0, Alu.add, Alu.subtract)),
                [t_dnet, zer_t]))

        s1, s2 = B[4], B[5]
        Sbuf, Mb, CB = F[2], F[1], F[3]     # swe, B/m1/M, cap/beta (f32)
        t_S = []
        t_M = None
        for it in range(SNOW_ITERS):
            if it == 0:
                tx = S.add("v", lambda: v.tensor_scalar(
                    out=s1[:], in0=B[1][:], scalar1=-1.0, scalar2=0.0,
                    op0=Alu.mult, op1=Alu.min), [t_dnet])
                tb = S.add("v", lambda: v.tensor_tensor(
                    s1[:], s1[:], B[0][:], Alu.add), [tx, t_snowf])
            else:
                tt1 = S.add("v", lambda: v.tensor_tensor(
                    body(s1), shifted(Mb), B[1][:, 1:FREE], Alu.add),
                    [t_M])
                tt1b = S.add("v", lambda: v.tensor_scalar(
                    out=cols0(s1), in0=v3(B[1][:])[:, :, 0], scalar1=0.001,
                    scalar2=0.0, op0=Alu.add, op1=Alu.add), [t_M])
                tx = S.add("v", lambda: v.scalar_tensor_tensor(
                    s1[:], s1[:], 0.0, B[1][:], Alu.min, Alu.subtract),
                    [tt1, tt1b])
                tb = S.add("v", lambda: v.tensor_tensor(
                    s1[:], s1[:], B[0][:], Alu.add), [tx])
            t_S = []
            for j in range(NCH):
                sl = slice(j * NSTEP, (j + 1) * NSTEP)
                t_S.append(S.add("v", (lambda sl=sl: v.tensor_tensor_scan(
                    Sbuf[:, sl], s1[:, sl], zb, 0.001, Alu.add, Alu.max)),
                    [tb]))
            # cap = cwh * S -> CB (pool, f32)
            tcap = S.add("p", lambda: gp.tensor_tensor(
                v3(CB[:]), v3(Sbuf[:]), pb("cwh"), Alu.mult),
                [t_prm] + t_S + ([t_M] if t_M else t_A))
            # B-scan(dnet, cap) -> Mb
            t_B = []
            for j in range(NCH):
                sl = slice(j * NSTEP, (j + 1) * NSTEP)
                t_B.append(S.add("v", (lambda sl=sl: v.tensor_tensor_scan(
                    Mb[:, sl], B[1][:, sl], CB[:, sl], INF,
                    Alu.add, Alu.min)), [tcap]))
            # beta = cap + A' (in place, pool)
            tbeta = S.add("p", lambda: gp.tensor_tensor(
                CB[:], CB[:], Ap[:], Alu.add), t_B + t_A)
            # m1 = min(-A', B) in place
            tm1 = S.add("v", lambda: v.scalar_tensor_tensor(
                Mb[:], Ap[:], -1.0, Mb[:], Alu.mult, Alu.min), t_B)
            # per chain: R-scan -> PSUM ; C = R - A' ; M = max(m1, C)
            t_Mj = []
            tprev = None
            for j in range(NCH):
                sl = slice(j * NSTEP, (j + 1) * NSTEP)
                trs = S.add("v", (lambda sl=sl: v.tensor_tensor_scan(
                    PS[:], CB[:, sl], Ap[:, sl], -INF,
                    Alu.min, Alu.max)), [tbeta, tm1, tprev])
                tcc = S.add("v", (lambda sl=sl: v.tensor_tensor(
                    PS[:], PS[:], Ap[:, sl], Alu.subtract)), [trs])
                tprev = S.add("v", (lambda sl=sl: v.tensor_tensor(
                    Mb[:, sl], Mb[:, sl], PS[:], Alu.max)), [tcc])
                t_Mj.append(tprev)
            t_M = t_Mj[-1]
        _tap("mwx", Mb[:], t_Mj, f32=True)
        # tosoil via water balance, f32 throughout the large totals
        ttwa = S.add("v", lambda: v.tensor_tensor(
            CB[:], Sbuf[:], Mb[:], Alu.add), t_Mj + [tbeta])
        tdt = S.add("v", lambda: v.tensor_tensor(
            body(s2), shifted(CB), body(CB), Alu.subtract), [ttwa])
        tdtb = S.add("v", lambda: v.tensor_scalar(
            out=cols0(s2), in0=cols0(CB), scalar1=-1.0, scalar2=0.002,
            op0=Alu.mult, op1=Alu.add), [ttwa])
        ttos = S.add("v", lambda: v.tensor_tensor(
            s2[:], s2[:], B[0][:], Alu.add), [tdt, tdtb])
        ttosc = S.add("v", lambda: v.tensor_scalar(
            out=s2[:], in0=s2[:], scalar1=0.0, scalar2=0.0,
            op0=Alu.max, op1=Alu.add), [ttos])
        _tap("twa", CB[:], [ttwa], f32=True)
        _tap("dtx", s2[:], [ttos])
        _tap("tos", s2[:], [ttosc])
        t_dswe = (S.dma(d_swe[:], Sbuf[:], t_S + [ttwa]) if tap is None
                  else S.dma(d_dbg32[:], Sbuf[:], t_S + [ttwa]))

        # ---------------- soil inputs into freed slots ----------------
        t_rainf = S.dma(B[3][:], din["rainf"][:])       # Mbuf slot
        t_petf = S.dma(B[0][:], din["petf"][:], [ttos])         # SNOWF slot
        t_kpe = S.dma(B[1][:], din["kpe"][:], [t_M, tx])        # DNET slot

        ttosf = S.add("v", lambda: v.scalar_tensor_tensor(
            v3(s1[:]), v3(s2[:]), 0.0, pb("invfc"), Alu.add, Alu.mult),
            [ttosc, ttwa])
        tinsf = S.add("v", lambda: v.tensor_tensor(
            B[3][:], B[3][:], s1[:], Alu.add), [ttosf, t_rainf])
        _tap("insf", B[3][:], [tinsf])

        # ================= SOIL =================
        PETF, KPE, INSF = B[0], B[1], B[3]
        ET, RCb, MSK = B[2], B[4], B[5]
        sig = F[0]
        tsig0 = S.add("v", lambda: v.memset(sig[:], 0.0), [t_M, ttwa])
        tsig = tsig0
        tscan = tsig0

        tirf = None
        for it, step in enumerate(SM_SCHED):
            newton = (step == "N")
            first = (it == 0)
            if first:
                # sig = 0 guess: w = 0 -> ET = 0
                tet = S.add("v", lambda: v.memset(ET[:], 0.0), [])
                trc = None
            else:
                trl = S.add("a", lambda: sc.activation(
                    body(F[1]), shifted(sig), Act.Relu),
                    [tscan, ttwa, t_dswe])
                trlb = S.add("a", lambda: sc.activation(
                    cols0(F[1]), prm_col("sig0"), Act.Relu),
                    [tscan, t_prm])
                tlt = S.add("a", lambda: sc.activation(
                    F[2][:], F[1][:], Act.Ln, bias=EPS[:], scale=1.0),
                    [trl, trlb, eps_t, t_dswe])
                tlb = S.add("p", lambda: gp.tensor_tensor(
                    v3(F[1][:]), v3(F[2][:]), pb("beta"), Alu.mult), [tlt])
                tet = S.add("a", lambda: sc.activation(
                    ET[:], F[1][:], Act.Exp), [tlb])
                trc = S.add("a", lambda: sc.activation(
                    RCb[:], F[2][:], Act.Exp, scale=-1.0),
                    [tlb]) if newton else None
            if newton:
                tmsk = S.add("v", lambda: v.tensor_scalar(
                    out=MSK[:], in0=ET[:], scalar1=1.0, scalar2=1.0,
                    op0=Alu.is_lt, op1=Alu.mult), [tet])
            tetm = S.add("v", lambda: v.tensor_scalar(
                out=ET[:], in0=ET[:], scalar1=1.0, scalar2=0.0,
                op0=Alu.min, op1=Alu.add),
                [tet] + ([tmsk] if newton else []))
            if newton:
                # P1 = insf*beta*w*msk  (in MSK)
                twe = S.add("v", lambda: v.tensor_tensor(
                    MSK[:], ET[:], MSK[:], Alu.mult), [tetm])
                twb = S.add("v", lambda: v.scalar_tensor_tensor(
                    v3(MSK[:]), v3(MSK[:]), 0.0, pb("beta"),
                    Alu.add, Alu.mult), [twe])
                tp1 = S.add("v", lambda: v.tensor_tensor(
                    MSK[:], MSK[:], INSF[:], Alu.mult), [twb, tinsf])
                # T = P1 / sig_prev = P1 * RC  (in RCb)
                tT = S.add("v", lambda: v.tensor_tensor(
                    RCb[:], MSK[:], RCb[:], Alu.mult), [tp1, trc])
            # IRF = min(w,1)*insf ; i = insf - IRF  (both through ET slot)
            tirf = S.add("v", lambda: v.tensor_tensor(
                ET[:], ET[:], INSF[:], Alu.mult),
                [tetm, tinsf] + ([twe] if newton else []))
            ti = S.add("v", lambda: v.tensor_tensor(
                ET[:], INSF[:], ET[:], Alu.subtract), [tirf])
            # sigSMP = sig_prev + i -> F[3]
            tsmp = S.add("p", lambda: gp.tensor_tensor(
                body(F[3]), shifted(sig), body(ET), Alu.add),
                [ti, tscan] + ([tdt, tdtb] if it == 0 else []))
            tsmpb = S.add("p", lambda: gp.tensor_tensor(
                cols0(F[3]), prm_col("sig0"), v3(ET[:])[:, :, 0], Alu.add),
                [ti, t_prm])
            if newton:
                # Z1 = i + P1 (in ET) ; then MSK free for MB
                tz1 = S.add("v", lambda: v.tensor_tensor(
                    ET[:], ET[:], MSK[:], Alu.add), [ti, tT, tsmp, tsmpb])
            else:
                tz1 = ti
            tmb = S.add("v", lambda: v.scalar_tensor_tensor(
                v3(MSK[:]), v3(F[3][:]), 0.0, pb("lp"), Alu.add, Alu.is_le),
                [tsmp, tsmpb, tz1])
            # V1 = MB*kpe -> F[1] (f32 coeff path)
            tv1 = S.add("p", lambda: gp.tensor_tensor(
                F[1][:], MSK[:], KPE[:], Alu.mult), [tmb, t_kpe])
            # VZ = V1*Z1 -> F[2]
            tvz = S.add("p", lambda: gp.tensor_tensor(
                F[2][:], F[1][:], ET[:], Alu.mult), [tv1, tz1])
            # b1 = Z1 - VZ -> ET
            tb1 = S.add("v", lambda: v.tensor_tensor(
                ET[:], ET[:], F[2][:], Alu.subtract), [tvz])
            # W1 = (MB-1)*petf -> MSK (MB dead after V1)
            tw1 = S.add("v", lambda: v.scalar_tensor_tensor(
                MSK[:], MSK[:], -1.0, PETF[:], Alu.add, Alu.mult),
                [tv1, t_petf])
            # b = b1 + W1 -> ET
            tbf = S.add("v", lambda: v.tensor_tensor(
                ET[:], ET[:], MSK[:], Alu.add), [tb1, tw1])
            # J: S3 chain
            if newton:
                tu = S.add("p", lambda: gp.tensor_tensor(
                    F[3][:], F[1][:], RCb[:], Alu.mult), [tvz, tT, tmb])
                ts3a = S.add("p", lambda: gp.tensor_tensor(
                    F[1][:], F[1][:], RCb[:], Alu.add), [tu])
                ts3 = S.add("p", lambda: gp.tensor_tensor(
                    F[1][:], F[1][:], F[3][:], Alu.subtract), [ts3a])
                jslot = F[3]
                tj = S.add("v", lambda: v.tensor_scalar(
                    out=F[3][:], in0=F[1][:], scalar1=-1.0, scalar2=1.0,
                    op0=Alu.mult, op1=Alu.add), [ts3])
            else:
                jslot = F[2]
                tj = S.add("v", lambda: v.tensor_scalar(
                    out=F[2][:], in0=F[1][:], scalar1=-1.0, scalar2=1.0,
                    op0=Alu.mult, op1=Alu.add), [tv1, tvz])
            # b[chain starts] += J0 * sig0
            tfix1 = S.add("v", lambda: v.tensor_tensor(
                cols0(MSK), v3(jslot[:])[:, :, 0], prm_col("sig0"),
                Alu.mult), [tj, tbf])
            tfix2 = S.add("v", lambda: v.tensor_tensor(
                cols0(ET), cols0(ET), cols0(MSK), Alu.add), [tfix1])
            t_sc = []
            for j in range(NCH):
                sl = slice(j * NSTEP, (j + 1) * NSTEP)
                t_sc.append(S.add("v", (lambda sl=sl, js=jslot:
                    v.tensor_tensor_scan(
                        sig[:, sl], js[:, sl], ET[:, sl], 0.0,
                        Alu.mult, Alu.add)), [tbf, tfix2, tj]))
            tscan = t_sc[-1]
            tscan_all = list(t_sc)
            _tap(f"sig{it}", sig[:], t_sc, f32=True)

        # ---------------- final soil outputs ----------------
        trl = S.add("a", lambda: sc.activation(
            body(F[1]), shifted(sig), Act.Relu), tscan_all)
        trlb = S.add("a", lambda: sc.activation(
            cols0(F[1]), prm_col("sig0"), Act.Relu), tscan_all)
        tlt = S.add("a", lambda: sc.activation(
            F[2][:], F[1][:], Act.Ln, bias=EPS[:], scale=1.0), [trl, trlb])
        tlb = S.add("p", lambda: gp.tensor_tensor(
            v3(F[1][:]), v3(F[2][:]), pb("beta"), Alu.mult),
            [tlt] + tscan_all)
        tet = S.add("a", lambda: sc.activation(
            ET[:], F[1][:], Act.Exp), [tlb])
        tetm = S.add("v", lambda: v.tensor_scalar(
            out=ET[:], in0=ET[:], scalar1=1.0, scalar2=0.0,
            op0=Alu.min, op1=Alu.add), [tet])
        tirf = S.add("v", lambda: v.tensor_tensor(
            RCb[:], ET[:], INSF[:], Alu.mult), [tetm])
        ti = S.add("v", lambda: v.tensor_tensor(
            MSK[:], INSF[:], RCb[:], Alu.subtract), [tirf])
        tsmp = S.add("p", lambda: gp.tensor_tensor(
            body(F[3]), shifted(sig), body(MSK), Alu.add), [ti, tlb])
        tsmpb = S.add("p", lambda: gp.tensor_tensor(
            cols0(F[3]), prm_col("sig0"), v3(MSK[:])[:, :, 0], Alu.add),
            [ti])
        tu4 = S.add("v", lambda: v.scalar_tensor_tensor(
            v3(ET[:]), v3(F[3][:]), 0.0, pb("invlp"), Alu.add, Alu.mult),
            [tsmp, tsmpb])
        tpet = S.add("v", lambda: v.scalar_tensor_tensor(
            v3(PETF[:]), v3(PETF[:]), 0.0, pb("fc"), Alu.add, Alu.mult),
            [tu4])
        taet = S.add("v", lambda: v.scalar_tensor_tensor(
            ET[:], ET[:], 1.0, PETF[:], Alu.min, Alu.mult), [tpet])
        _tap("aetx", ET[:], [taet])
        t_g = S.dma(F[2][:], din["gsc"][:], [tlb, tlt])
        t_daet = S.dma(d_aet[:] if tap is None else d_dbg16[:], ET[:],
                       [taet])
        tsm1 = S.add("p", lambda: gp.tensor_tensor(
            body(F[1]), shifted(sig), body(INSF), Alu.add), [ti, tet, tsmp])
        tsm1b = S.add("p", lambda: gp.tensor_tensor(
            cols0(F[1]), prm_col("sig0"), v3(INSF[:])[:, :, 0], Alu.add),
            [ti, tet])
        trr1 = S.add("v", lambda: v.tensor_scalar(
            out=MSK[:], in0=F[1][:], scalar1=-1.0, scalar2=1.0,
            op0=Alu.add, op1=Alu.mult), [tsm1, tsm1b, tu4])
        trr2 = S.add("v", lambda: v.tensor_tensor(
            MSK[:], MSK[:], RCb[:], Alu.max), [trr1])
        trin = S.add("v", lambda: v.scalar_tensor_tensor(
            v3(MSK[:]), v3(MSK[:]), 0.0, pb("fc"), Alu.add, Alu.mult),
            [trr2])
        RIN = MSK
        _tap("rin", MSK[:], [trin])

        # ================= SUZ + SLZ =================
        t_h = S.dma(F[3][:], din["hsc"][:], [tu4])
        trp = S.add("v", lambda: v.scalar_tensor_tensor(
            v3(RCb[:]), v3(RIN[:]), 0.0, pb("pmax"),
            Alu.add, Alu.subtract), [trin])
        # BZ = rp * G -> F[1] (f32; scaled magnitudes need f32)
        tbz = S.add("v", lambda: v.tensor_tensor(
            F[1][:], RCb[:], F[2][:], Alu.mult), [trp, t_g, trr1, tsm1b])
        # z-scans -> F[0] (sig dead); chunk carries in CAR (f32 tiny)
        t_car = None
        t_zs_all = []
        for c in range(NCHUNK):
            t_zs = []
            for j in range(NCH):
                lo = j * NSTEP + c * CHUNK
                sl = slice(lo, lo + CHUNK)
                if c == 0:
                    t_zs.append(S.add("v", (lambda sl=sl:
                        v.tensor_tensor_scan(
                            sig[:, sl], F[1][:, sl],
                            ZERO[:].broadcast_to([128, CHUNK]), 0.001,
                            Alu.add, Alu.max)), [tbz, tsm1, tsmp]))
                else:
                    ini = CAR[:, (c - 1) * NCH + j:(c - 1) * NCH + j + 1]
                    t_zs.append(S.add("v", (lambda sl=sl, ini=ini:
                        v.tensor_tensor_scan(
                            sig[:, sl], F[1][:, sl],
                            ZERO[:].broadcast_to([128, CHUNK]), ini,
                            Alu.add, Alu.max)), [tbz, t_car]))
            lastc = c * CHUNK + CHUNK - 1
            t_car = S.add("v", (lambda c=c, lastc=lastc: v.tensor_tensor(
                CAR[:, c * NCH:(c + 1) * NCH], v3(sig[:])[:, :, lastc],
                v3(F[3][:])[:, :, lastc], Alu.mult)), t_zs + [t_h])
            t_zs_all += t_zs
        # SUPREV -> F[1] (overwrites BZ): body = z_prev * H_prev; col0 .001
        tsup = S.add("v", lambda: v.tensor_tensor(
            body(F[1]), shifted(sig), shifted(F[3]), Alu.mult),
            t_zs_all + [t_car])
        tsupb = S.add("v", lambda: v.memset(cols0(F[1]), 0.001), [tbz])
        _tap("su", F[1][:], [tsup, tsupb], f32=True)
        # pu = max(SUPREV + rp, 0) -> B[1] ; su1 = SUPREV + rin -> B[0]
        tpu1 = S.add("v", lambda: v.tensor_tensor(
            B[1][:], F[1][:], RCb[:], Alu.add), [tsup, tsupb])
        tsu1 = S.add("v", lambda: v.tensor_tensor(
            B[0][:], F[1][:], RIN[:], Alu.add),
            [tsup, tsupb, taet, tpet])
        tpu = S.add("v", lambda: v.tensor_scalar(
            out=B[1][:], in0=B[1][:], scalar1=0.0, scalar2=0.0,
            op0=Alu.max, op1=Alu.add), [tpu1])
        tperc = S.add("v", lambda: v.scalar_tensor_tensor(
            v3(B[0][:]), v3(B[0][:]), 0.0, pb("pmax"), Alu.add, Alu.min),
            [tsu1])
        _tap("perc", B[0][:], [tperc])
        # q0 = k0*relu(pu - uzl) -> B[2] ; q01 -> RCb
        td = S.add("v", lambda: v.scalar_tensor_tensor(
            v3(B[2][:]), v3(B[1][:]), 0.0, pb("uzl"), Alu.add,
            Alu.subtract), [tpu, t_daet])
        tr0 = S.add("v", lambda: v.tensor_scalar(
            out=B[2][:], in0=B[2][:], scalar1=0.0, scalar2=0.0,
            op0=Alu.max, op1=Alu.add), [td])
        tq0 = S.add("v", lambda: v.scalar_tensor_tensor(
            v3(B[2][:]), v3(B[2][:]), 0.0, pb("k0"), Alu.add, Alu.mult),
            [tr0])
        tw1q = S.add("v", lambda: v.tensor_tensor(
            RCb[:], B[1][:], B[2][:], Alu.subtract), [tq0, trp, tbz])
        tq1 = S.add("v", lambda: v.scalar_tensor_tensor(
            v3(RCb[:]), v3(RCb[:]), 0.0, pb("k1"), Alu.add, Alu.mult),
            [tw1q])
        tq01 = S.add("v", lambda: v.tensor_tensor(
            RCb[:], RCb[:], B[2][:], Alu.add), [tq1])
        _tap("q01", RCb[:], [tq01])
        # SLZ scan(perc, omk2) -> B[1] ; q2 ; Q
        t_slz = []
        for j in range(NCH):
            sl = slice(j * NSTEP, (j + 1) * NSTEP)
            omb = prm_col("omk2")[:, j:j + 1].broadcast_to([128, NSTEP])
            t_slz.append(S.add("v", (lambda sl=sl, omb=omb:
                v.tensor_tensor_scan(
                    B[1][:, sl], B[0][:, sl], omb, 0.001,
                    Alu.add, Alu.mult)), [tperc, tq01]))
        tq2 = S.add("v", lambda: v.scalar_tensor_tensor(
            v3(B[1][:]), v3(B[1][:]), 0.0, pb("kq2"), Alu.add, Alu.mult),
            t_slz)
        tq = S.add("v", lambda: v.tensor_tensor(
            B[1][:], B[1][:], RCb[:], Alu.add), [tq2])
        S.dma(d_q[:] if tap is None else d_dbg16[:], B[1][:], [tq])

        # ================= emit =================
        def emit_queue(queue, eng):
            hw = {}
            for emit, waits in S.ops[eng]:
                for de, dc in waits.items():
                    if hw.get(de, 0) >= dc:
                        continue
                    if de == "d":
                        queue.wait_ge(dsem, 16 * dc)
                    else:
                        queue.wait_ge(sem[de], dc)
                    hw[de] = dc
                emit().then_inc(sem[eng], 1)

        @block.sync
        def _(sync):
            hw = {}
            for dst, src, deps in S.dmas:
                for de, dc in deps:
                    if de == "d" or hw.get(de, 0) >= dc:
                        continue
                    sync.wait_ge(sem[de], dc)
                    hw[de] = dc
                sync.dma_start(out=dst, in_=src).then_inc(dsem, 16)
            sync.wait_ge(dsem, 16 * S.n_dma)

        @block.vector
        def _(vector):
            emit_queue(vector, "v")

        @block.scalar
        def _(scalar):
            emit_queue(scalar, "a")

        @block.gpsimd
        def _(g):
            emit_queue(g, "p")

    return nc


def _pack_series(a):
    """[730, 1280] -> [128, 7300] chain-major (free = j*730 + t)."""
    return np.ascontiguousarray(
        a.reshape(NSTEP, NCH, 128).transpose(2, 1, 0).reshape(128, FREE))


def _pack_param(a):
    return np.ascontiguousarray(a.reshape(NCH, 128).T)


def _unpack_series(a):
    return np.asarray(a, dtype=np.float32).reshape(128, NCH, NSTEP).transpose(
        2, 1, 0).reshape(NSTEP, PADG)[:, :CELLS]


def _prep_core(xs, ps):
    npad = PADG - xs.shape[1]
    xs = np.concatenate([xs, np.repeat(xs[:, :1, :], npad, axis=1)], axis=1)
    ps = np.concatenate([ps, np.repeat(ps[:1, :], npad, axis=0)], axis=0)
    ps = ps.astype(np.float64)
    BETA, FC, K0, K1, K2, LP, PERCmax, UZL, TT, CFMAX, CFR, CWH = (
        ps[:, i] for i in range(12))
    P = xs[:, :, 0].astype(np.float64)
    T = xs[:, :, 1].astype(np.float64)
    PET = xs[:, :, 2].astype(np.float64)
    is_rain = T >= TT[None, :]
    RAIN = np.where(is_rain, P, 0.0)
    SNOWF = np.where(is_rain, 0.0, P)
    DNET = np.maximum(CFMAX[None, :] * (T - TT[None, :]), 0.0) - \
        np.maximum((CFR * CFMAX)[None, :] * (TT[None, :] - T), 0.0)
    LPFC = LP * FC
    KPE = PET / LPFC[None, :]
    RAINF = RAIN / FC[None, :]
    PETF = PET / FC[None, :]
    OMK1 = 1.0 - K1
    ell = np.arange(CHUNK)
    G1 = OMK1[:, None] ** (-ell)[None, :]          # [1280, 73]
    H1 = OMK1[:, None] ** (ell + 1)[None, :]
    G = np.tile(G1, (1, NCHUNK)).T                 # [730, 1280]
    H = np.tile(H1, (1, NCHUNK)).T

    bf = ml_dtypes.bfloat16
    prm_parts = {
        "cwh": CWH, "beta": BETA, "invfc": 1.0 / FC, "invlp": 1.0 / LP,
        "fc": FC, "lp": LP, "pmax": PERCmax, "uzl": UZL, "k0": K0, "k1": K1,
        "kq2": K2 / (1.0 - K2), "omk2": 1.0 - K2,
        "sig0": 0.001 / FC,
    }
    prm = np.concatenate(
        [_pack_param(prm_parts[n].astype(np.float32)) for n in PARAM_NAMES],
        axis=1)
    return {
        "snowf": _pack_series(SNOWF.astype(np.float32)).astype(bf),
        "dnet": _pack_series(DNET.astype(np.float32)).astype(bf),
        "rainf": _pack_series(RAINF.astype(np.float32)).astype(bf),
        "petf": _pack_series(PETF.astype(np.float32)).astype(bf),
        "kpe": _pack_series(KPE.astype(np.float32)).astype(bf),
        "gsc": _pack_series(G.astype(np.float32)),
        "hsc": _pack_series(H.astype(np.float32)),
        "prm": np.ascontiguousarray(prm.astype(np.float32)),
    }


def kernel(x, parameters):
    global LAST_RESULT, _NC
    x = np.asarray(x, dtype=np.float32)
    parameters = np.asarray(parameters, dtype=np.float32)
    lo = BOUNDS[:, 0][None, :]
    hi = BOUNDS[:, 1][None, :]
    ps_all = (lo + parameters[-1, :, :, 0] * (hi - lo)).astype(np.float32)

    in_maps = []
    for c in range(NCORE):
        sl = slice(c * CELLS, (c + 1) * CELLS)
        in_maps.append(_prep_core(x[:, sl, :], ps_all[sl]))

    if _NC is None:
        _NC = _build()
    res = run_bass_kernel_spmd(_NC, in_maps, core_ids=list(range(NCORE)))
    LAST_RESULT = res

    q = np.empty((NSTEP, NGRID), np.float32)
    aet = np.empty((NSTEP, NGRID), np.float32)
    swe = np.empty((NSTEP, NGRID), np.float32)
    for c in range(NCORE):
        sl = slice(c * CELLS, (c + 1) * CELLS)
        out = res.results[c]
        q[:, sl] = _unpack_series(out["q"])
        aet[:, sl] = _unpack_series(out["aet"])
        swe[:, sl] = _unpack_series(out["swe"])
    return q[:, :, None], aet[:, :, None], swe[:, :, None]
